# revision 3
# baseline (speedup 1.0000x reference)
"""AttentionBlock (GroupNorm + single-head self-attention + proj + residual) on 8 Trainium2
NeuronCores, data-parallel over the batch (16 samples -> 2 per core).

Per-sample math (C=512 channels, N=1024 tokens = 32x32 spatial):
  h   = GroupNorm(x; 8 groups) * w + b           [C, N]
  q,k = Wq@h + bq, Wk@h + bk                     [C, N]   (c on partitions)
  vT  = h^T @ WvT + bv(deferred)                 [N, C]   (tokens on partitions)
  S^T = k^T q * scale                            [j, i]   (j on partitions)
  E   = exp(S^T)       den[i] = sum_j E[j,i]     (den via ones-matmuls, i on partitions)
  oT  = (E^T-contraction) : oT[i,c] = sum_j E[j,i] vT[j,c];  oT *= 1/den[i]
  out = transpose(oT) + bv                       [C, N]
  y   = x + Wp@out + bp                          [C, N]

All matmul operands are bf16 (PSUM accumulates fp32); softmax/statistics are fp32.
Softmax skips max-subtraction: scores are ~N(0,1) (|s| < ~7), exp() is safe in fp32.
"""

import numpy as np
import ml_dtypes

import concourse.bacc as bacc
import concourse.tile as tile
from concourse import mybir
from concourse.bass_utils import run_bass_kernel_spmd
from concourse.masks import make_identity

F32 = mybir.dt.float32
BF16 = mybir.dt.bfloat16
AF = mybir.ActivationFunctionType
OP = mybir.AluOpType

NCORES = 8
S = 2          # samples per core
C = 512
N = 1024       # H*W
CT = C // 128  # channel tiles
NT = N // 128  # token tiles
EPS = 1e-5
SCALE = float(C) ** -0.5


def build_nc():
    nc = bacc.Bacc("TRN2", target_bir_lowering=False)
    x_d = nc.dram_tensor("x", [S, C, N], F32, kind="ExternalInput")
    wT_d = nc.dram_tensor("qkv_wT", [C, 3 * C], BF16, kind="ExternalInput")
    pwT_d = nc.dram_tensor("proj_wT", [C, C], BF16, kind="ExternalInput")
    nw_d = nc.dram_tensor("norm_w", [C], F32, kind="ExternalInput")
    nb_d = nc.dram_tensor("norm_b", [C], F32, kind="ExternalInput")
    qkvb_d = nc.dram_tensor("qkv_b", [3 * C], F32, kind="ExternalInput")
    pb_d = nc.dram_tensor("proj_b", [C], F32, kind="ExternalInput")
    gm_d = nc.dram_tensor("gmat", [128, 128], F32, kind="ExternalInput")
    out_d = nc.dram_tensor("out", [S, C, N], F32, kind="ExternalOutput")

    with tile.TileContext(nc) as tc:
        with (
            tc.tile_pool(name="consts", bufs=1) as consts,
            tc.tile_pool(name="xp", bufs=2) as xp,
            tc.tile_pool(name="hp", bufs=2) as hp,
            tc.tile_pool(name="qp", bufs=2) as qp,
            tc.tile_pool(name="kp", bufs=2) as kp,
            tc.tile_pool(name="vp", bufs=2) as vp,
            tc.tile_pool(name="esp", bufs=1) as esp,
            tc.tile_pool(name="otp", bufs=1) as otp,
            tc.tile_pool(name="aop", bufs=1) as aop,
            tc.tile_pool(name="finp", bufs=2) as finp,
            tc.tile_pool(name="statp", bufs=4) as statp,
            tc.tile_pool(name="ps_big", bufs=2, space="PSUM") as ps_big,
            tc.tile_pool(name="ps_mid", bufs=2, space="PSUM") as ps_mid,
            tc.tile_pool(name="ps_sm", bufs=1, space="PSUM") as ps_sm,
        ):
            # ---------------- constants ----------------
            wT = consts.tile([128, CT, 3 * C], BF16, tag="wT")
            wT_r = wT_d.ap().rearrange("(kc p) o -> p kc o", p=128)
            for kc in range(CT):
                nc.sync.dma_start(wT[:, kc, :], wT_r[:, kc, :])
            pwT = consts.tile([128, CT, C], BF16, tag="pwT")
            pwT_r = pwT_d.ap().rearrange("(kc p) o -> p kc o", p=128)
            for kc in range(CT):
                nc.sync.dma_start(pwT[:, kc, :], pwT_r[:, kc, :])
            gmat = consts.tile([128, 128], F32, tag="gmat")
            nc.sync.dma_start(gmat, gm_d.ap())
            ident = consts.tile([128, 128], F32, tag="ident")
            make_identity(nc, ident)
            ones_bf = consts.tile([128, 1], BF16, tag="ones")
            nc.vector.memset(ones_bf, 1.0)
            epsb = consts.tile([128, 1], F32, tag="eps")
            nc.vector.memset(epsb, EPS)
            nw = consts.tile([128, CT], F32, tag="nw")
            nc.sync.dma_start(nw, nw_d.ap().rearrange("(t p) -> p t", p=128))
            nb = consts.tile([128, CT], F32, tag="nb")
            nc.sync.dma_start(nb, nb_d.ap().rearrange("(t p) -> p t", p=128))
            qb = consts.tile([128, CT], F32, tag="qb")
            nc.sync.dma_start(qb, qkvb_d.ap()[0:C].rearrange("(t p) -> p t", p=128))
            kb = consts.tile([128, CT], F32, tag="kb")
            nc.sync.dma_start(kb, qkvb_d.ap()[C:2 * C].rearrange("(t p) -> p t", p=128))
            vb = consts.tile([128, CT], F32, tag="vb")
            nc.sync.dma_start(vb, qkvb_d.ap()[2 * C:3 * C].rearrange("(t p) -> p t", p=128))
            pb = consts.tile([128, CT], F32, tag="pb")
            nc.sync.dma_start(pb, pb_d.ap().rearrange("(t p) -> p t", p=128))

            for s in range(S):
                # ---------------- load x ----------------
                x_sb = xp.tile([128, CT, N], F32, tag="x")
                for ct in range(CT):
                    nc.sync.dma_start(x_sb[:, ct, :], x_d[s, ct * 128:(ct + 1) * 128, :])

                # ---------------- groupnorm ----------------
                h_sb = hp.tile([128, CT, N], BF16, tag="h")
                for ct in range(CT):
                    st = statp.tile([128, 2, 6], F32, tag="bnst")
                    for i in range(2):
                        nc.vector.bn_stats(st[:, i, :], x_sb[:, ct, i * 512:(i + 1) * 512])
                    mv = statp.tile([128, 2], F32, tag="mv")
                    nc.vector.bn_aggr(mv, st)
                    # columns (mean, E[x^2]) per partition for group aggregation
                    msq = statp.tile([128, 2], F32, tag="msq")
                    nc.vector.tensor_copy(msq[:, 0:1], mv[:, 0:1])
                    nc.vector.tensor_tensor(msq[:, 1:2], mv[:, 0:1], mv[:, 0:1], OP.mult)
                    nc.vector.tensor_tensor(msq[:, 1:2], msq[:, 1:2], mv[:, 1:2], OP.add)
                    # group-average and broadcast back to all 128 partitions (1 matmul)
                    gps = ps_sm.tile([128, 8], F32, tag="gnagg")
                    nc.tensor.matmul(gps[:, 0:2], lhsT=gmat, rhs=msq, start=True, stop=True)
                    gst = statp.tile([128, 2], F32, tag="gst")
                    nc.vector.tensor_copy(gst, gps[:, 0:2])
                    # scale = rstd * w ; shift = b - mean * scale
                    sc = statp.tile([128, 2], F32, tag="sc")
                    tmp = statp.tile([128, 1], F32, tag="tmp")
                    nc.vector.tensor_tensor(tmp, gst[:, 0:1], gst[:, 0:1], OP.mult)
                    nc.vector.tensor_tensor(tmp, gst[:, 1:2], tmp, OP.subtract)  # var
                    # rstd = exp(-0.5*ln(var+eps)); Ln+Exp live in one ACT table set
                    nc.scalar.activation(tmp, tmp, AF.Ln, bias=epsb, scale=1.0)
                    nc.scalar.activation(tmp, tmp, AF.Exp, bias=0.0, scale=-0.5)
                    nc.vector.tensor_tensor(sc[:, 0:1], tmp, nw[:, ct:ct + 1], OP.mult)
                    nc.vector.tensor_tensor(tmp, gst[:, 0:1], sc[:, 0:1], OP.mult)
                    nc.vector.tensor_tensor(sc[:, 1:2], nb[:, ct:ct + 1], tmp, OP.subtract)
                    nc.scalar.activation(h_sb[:, ct, :], x_sb[:, ct, :], AF.Identity,
                                         bias=sc[:, 1:2], scale=sc[:, 0:1])

                # ---------------- QKV ----------------
                q_sb = qp.tile([128, CT, N], BF16, tag="q")
                k_sb = kp.tile([128, CT, N], BF16, tag="k")
                vT_sb = vp.tile([128, NT, C], BF16, tag="vT")
                for mo in range(CT):
                    for nch in range(2):
                        ps = ps_mid.tile([128, 512], F32, tag="mid")
                        for kc in range(CT):
                            nc.tensor.matmul(ps, lhsT=wT[:, kc, mo * 128:(mo + 1) * 128],
                                             rhs=h_sb[:, kc, nch * 512:(nch + 1) * 512],
                                             start=(kc == 0), stop=(kc == CT - 1))
                        nc.scalar.activation(q_sb[:, mo, nch * 512:(nch + 1) * 512], ps,
                                             AF.Identity, bias=qb[:, mo:mo + 1], scale=1.0)
                for mo in range(CT):
                    for nch in range(2):
                        ps = ps_mid.tile([128, 512], F32, tag="mid")
                        for kc in range(CT):
                            nc.tensor.matmul(ps, lhsT=wT[:, kc, C + mo * 128:C + (mo + 1) * 128],
                                             rhs=h_sb[:, kc, nch * 512:(nch + 1) * 512],
                                             start=(kc == 0), stop=(kc == CT - 1))
                        nc.scalar.activation(k_sb[:, mo, nch * 512:(nch + 1) * 512], ps,
                                             AF.Identity, bias=kb[:, mo:mo + 1], scale=1.0)
                for it in range(NT):
                    ps = ps_mid.tile([128, 512], F32, tag="mid")
                    for kc in range(CT):
                        nc.tensor.matmul(ps, lhsT=h_sb[:, kc, it * 128:(it + 1) * 128],
                                         rhs=wT[:, kc, 2 * C:3 * C],
                                         start=(kc == 0), stop=(kc == CT - 1))
                    nc.vector.tensor_copy(vT_sb[:, it, :], ps)

                # ---------------- S^T, exp, den ----------------
                es_sb = esp.tile([128, NT, N], BF16, tag="es")
                den_ps = ps_sm.tile([128, 8], F32, tag="den")
                nc.vector.memset(den_ps, 0.0)
                for jt in range(NT):
                    ps = ps_big.tile([128, N], F32, tag="big")
                    for nch in range(2):
                        for kc in range(CT):
                            nc.tensor.matmul(ps[:, nch * 512:(nch + 1) * 512],
                                             lhsT=k_sb[:, kc, jt * 128:(jt + 1) * 128],
                                             rhs=q_sb[:, kc, nch * 512:(nch + 1) * 512],
                                             start=(kc == 0), stop=(kc == CT - 1))
                    nc.scalar.activation(es_sb[:, jt, :], ps, AF.Exp, bias=0.0, scale=SCALE)
                    # den[i] += sum_j(this tile): accumulate into a memset psum bank
                    # (start=False everywhere; first write per element overwrites or
                    # adds to the zeroed data -- correct either way)
                    for ic in range(NT):
                        nc.tensor.matmul(den_ps[:, ic:ic + 1],
                                         lhsT=es_sb[:, jt, ic * 128:(ic + 1) * 128],
                                         rhs=ones_bf,
                                         start=False, stop=False, skip_group_check=True)
                recip = statp.tile([128, 8], F32, tag="recip")
                nc.vector.reciprocal(recip, den_ps)

                # ---------------- AV (-> oT[i, c]) ----------------
                oT_sb = otp.tile([128, NT, C], F32, tag="oT")
                for it in range(NT):
                    ps = ps_mid.tile([128, 512], F32, tag="mid")
                    for jt in range(NT):
                        nc.tensor.matmul(ps, lhsT=es_sb[:, jt, it * 128:(it + 1) * 128],
                                         rhs=vT_sb[:, jt, :],
                                         start=(jt == 0), stop=(jt == NT - 1))
                    nc.vector.tensor_scalar(oT_sb[:, it, :], ps, recip[:, it:it + 1], None,
                                            OP.mult)

                # ---------------- transpose oT -> out[c, n] (+bv) ----------------
                ao_sb = aop.tile([128, CT, N], BF16, tag="ao")
                for ct in range(CT):
                    ps = ps_big.tile([128, N], F32, tag="big")
                    for it in range(NT):
                        nc.tensor.transpose(ps[:, it * 128:(it + 1) * 128],
                                            oT_sb[:, it, ct * 128:(ct + 1) * 128], ident)
                    nc.scalar.activation(ao_sb[:, ct, :], ps, AF.Identity,
                                         bias=vb[:, ct:ct + 1], scale=1.0)

                # ---------------- proj + residual ----------------
                fin_sb = finp.tile([128, CT, N], F32, tag="fin")
                for ct in range(CT):
                    nc.gpsimd.tensor_scalar(fin_sb[:, ct, :], x_sb[:, ct, :],
                                            pb[:, ct:ct + 1], None, OP.add)
                for mo in range(CT):
                    for nch in range(2):
                        ps = ps_mid.tile([128, 512], F32, tag="mid")
                        for kc in range(CT):
                            nc.tensor.matmul(ps, lhsT=pwT[:, kc, mo * 128:(mo + 1) * 128],
                                             rhs=ao_sb[:, kc, nch * 512:(nch + 1) * 512],
                                             start=(kc == 0), stop=(kc == CT - 1))
                        sl = fin_sb[:, mo, nch * 512:(nch + 1) * 512]
                        nc.vector.tensor_tensor(sl, ps, sl, OP.add)
                for ct in range(CT):
                    nc.sync.dma_start(out_d[s, ct * 128:(ct + 1) * 128, :], fin_sb[:, ct, :])

    nc.finalize()
    return nc


_NC_CACHE = None
LAST_EXEC_NS = None
LAST_RESULTS = None


def _get_nc():
    global _NC_CACHE
    if _NC_CACHE is None:
        _NC_CACHE = build_nc()
    return _NC_CACHE


def make_gmat():
    g = np.zeros((128, 128), np.float32)
    g[:64, :64] = 1.0 / 64
    g[64:, 64:] = 1.0 / 64
    return g


def make_in_maps(x, norm_w, norm_b, qkv_w, qkv_b, proj_w, proj_b):
    bf = ml_dtypes.bfloat16
    x = np.asarray(x, np.float32)
    B = x.shape[0]
    x_r = np.ascontiguousarray(x.reshape(B, C, N))
    qkv_wT = np.ascontiguousarray(np.asarray(qkv_w, np.float32).T).astype(bf)
    proj_wT = np.ascontiguousarray(np.asarray(proj_w, np.float32).T).astype(bf)
    common = {
        "qkv_wT": qkv_wT,
        "proj_wT": proj_wT,
        "norm_w": np.ascontiguousarray(np.asarray(norm_w, np.float32)),
        "norm_b": np.ascontiguousarray(np.asarray(norm_b, np.float32)),
        "qkv_b": np.ascontiguousarray(np.asarray(qkv_b, np.float32)),
        "proj_b": np.ascontiguousarray(np.asarray(proj_b, np.float32)),
        "gmat": make_gmat(),
    }
    per = B // NCORES
    return [dict(common, x=np.ascontiguousarray(x_r[c * per:(c + 1) * per]))
            for c in range(NCORES)]


def kernel(x, norm_w, norm_b, qkv_w, qkv_b, proj_w, proj_b, _trace=False):
    global LAST_EXEC_NS, LAST_RESULTS
    x = np.asarray(x)
    B, C_, H, W = x.shape
    in_maps = make_in_maps(x, norm_w, norm_b, qkv_w, qkv_b, proj_w, proj_b)
    res = run_bass_kernel_spmd(_get_nc(), in_maps, core_ids=list(range(NCORES)),
                               trace=_trace)
    LAST_EXEC_NS = res.exec_time_ns
    LAST_RESULTS = res
    out = np.concatenate([res.results[c]["out"] for c in range(NCORES)], axis=0)
    return out.reshape(B, C_, H, W).astype(np.float32)


# revision 5
# speedup vs baseline: 1.8316x; 1.8316x over previous
"""AttentionBlock (GroupNorm + single-head self-attention + proj + residual) on 8 Trainium2
NeuronCores, data-parallel over the batch (16 samples -> 2 per core).

Per-sample math (C=512 channels, N=1024 tokens = 32x32 spatial):
  h   = GroupNorm(x; 8 groups) * w + b           [C, N]
  q,k = Wq@h + bq, Wk@h + bk                     [C, N]   (c on partitions)
  vT  = h^T @ WvT (+bv deferred past softmax)    [N, C]   (tokens on partitions)
  S^T = k^T q * scale                            [j, i]   (j on partitions)
  E   = exp(S^T)       den[i] = sum_j E[j,i]     (den via ones-matmuls, i on partitions)
  oT[i,c] = sum_j E[j,i] vT[j,c];  oT *= 1/den[i]
  out = transpose(oT) + bv                       [C, N]
  y   = (x + bp) + Wp@out                        [C, N]

All matmul operands are bf16 (PSUM accumulates fp32); softmax/statistics are fp32.
Softmax skips max-subtraction: scores are ~N(0,1) (|s| < ~7), exp() is safe in fp32.
"""

import numpy as np
import ml_dtypes

import concourse.bacc as bacc
import concourse.tile as tile
from concourse import mybir
from concourse.bass_utils import run_bass_kernel_spmd
from concourse.hw_specs import get_activation_tables as _gat
from concourse.masks import make_identity

F32 = mybir.dt.float32
BF16 = mybir.dt.bfloat16
AF = mybir.ActivationFunctionType
OP = mybir.AluOpType

NCORES = 8
S = 2          # samples per core
C = 512
N = 1024       # H*W
CT = C // 128  # channel tiles
NT = N // 128  # token tiles
EPS = 1e-5
SCALE = float(C) ** -0.5

# All ACT funcs we use (Exp, Ln, Identity, Copy) live in one table set; blank out the
# other sets (keeping list positions!) so the table-load pass never alternates sets.
_ONE_SET = "natural_log_exp_and_others"


def _gat_filtered(arch):
    return {name: (fns if name == _ONE_SET else set())
            for name, fns in _gat(arch).items()}


bacc.get_activation_tables = _gat_filtered


def build_nc():
    nc = bacc.Bacc("TRN2", target_bir_lowering=False)
    x_d = nc.dram_tensor("x", [S, C, N], F32, kind="ExternalInput")
    wT_d = nc.dram_tensor("qkv_wT", [C, 3 * C], BF16, kind="ExternalInput")
    pwT_d = nc.dram_tensor("proj_wT", [C, C], BF16, kind="ExternalInput")
    nw_d = nc.dram_tensor("norm_w", [C], F32, kind="ExternalInput")
    nb_d = nc.dram_tensor("norm_b", [C], F32, kind="ExternalInput")
    qkvb_d = nc.dram_tensor("qkv_b", [3 * C], F32, kind="ExternalInput")
    pb_d = nc.dram_tensor("proj_b", [C], F32, kind="ExternalInput")
    gm_d = nc.dram_tensor("gmat", [128, 128], F32, kind="ExternalInput")
    out_d = nc.dram_tensor("out", [S, C, N], F32, kind="ExternalOutput")

    with tile.TileContext(nc) as tc:
        with (
            tc.tile_pool(name="consts", bufs=1) as consts,
            tc.tile_pool(name="xp", bufs=2) as xp,
            tc.tile_pool(name="hp", bufs=2) as hp,
            tc.tile_pool(name="qp", bufs=2) as qp,
            tc.tile_pool(name="kp", bufs=2) as kp,
            tc.tile_pool(name="vp", bufs=2) as vp,
            tc.tile_pool(name="esp", bufs=1) as esp,
            tc.tile_pool(name="otp", bufs=1) as otp,
            tc.tile_pool(name="aop", bufs=1) as aop,
            tc.tile_pool(name="finp", bufs=2) as finp,
            tc.tile_pool(name="statp", bufs=4) as statp,
            tc.tile_pool(name="ps_big", bufs=2, space="PSUM") as ps_big,
            tc.tile_pool(name="ps_mid", bufs=2, space="PSUM") as ps_mid,
            tc.tile_pool(name="ps_sm", bufs=1, space="PSUM") as ps_sm,
        ):
            # ---------------- constants ----------------
            wT = consts.tile([128, CT, 3 * C], BF16, tag="wT")
            wT_r = wT_d.ap().rearrange("(kc p) o -> p kc o", p=128)
            for kc in range(CT):
                nc.sync.dma_start(wT[:, kc, :], wT_r[:, kc, :])
            pwT = consts.tile([128, CT, C], BF16, tag="pwT")
            pwT_r = pwT_d.ap().rearrange("(kc p) o -> p kc o", p=128)
            for kc in range(CT):
                nc.sync.dma_start(pwT[:, kc, :], pwT_r[:, kc, :])
            gmat = consts.tile([128, 128], F32, tag="gmat")
            nc.sync.dma_start(gmat, gm_d.ap())
            ident = consts.tile([128, 128], F32, tag="ident")
            make_identity(nc, ident)
            ones_bf = consts.tile([128, 1], BF16, tag="ones")
            nc.vector.memset(ones_bf, 1.0)
            epsb = consts.tile([128, 1], F32, tag="eps")
            nc.vector.memset(epsb, EPS)
            nw = consts.tile([128, CT], F32, tag="nw")
            nc.sync.dma_start(nw, nw_d.ap().rearrange("(t p) -> p t", p=128))
            nb = consts.tile([128, CT], F32, tag="nb")
            nc.sync.dma_start(nb, nb_d.ap().rearrange("(t p) -> p t", p=128))
            qb = consts.tile([128, CT], F32, tag="qb")
            nc.sync.dma_start(qb, qkvb_d.ap()[0:C].rearrange("(t p) -> p t", p=128))
            kb = consts.tile([128, CT], F32, tag="kb")
            nc.sync.dma_start(kb, qkvb_d.ap()[C:2 * C].rearrange("(t p) -> p t", p=128))
            vb = consts.tile([128, CT], F32, tag="vb")
            nc.sync.dma_start(vb, qkvb_d.ap()[2 * C:3 * C].rearrange("(t p) -> p t", p=128))
            pb = consts.tile([128, CT], F32, tag="pb")
            nc.sync.dma_start(pb, pb_d.ap().rearrange("(t p) -> p t", p=128))

            x_sb, h_sb, q_sb, k_sb, vT_sb = {}, {}, {}, {}, {}
            es_sb, oT_sb, ao_sb, fin_sb, recip = {}, {}, {}, {}, {}

            # ---------------- load x + groupnorm ----------------
            for s in range(S):
                x_sb[s] = xp.tile([128, CT, N], F32, tag="x", name=f"x{s}")
                for ct in range(CT):
                    nc.sync.dma_start(x_sb[s][:, ct, :], x_d[s, ct * 128:(ct + 1) * 128, :])

            for s in range(S):
                h_sb[s] = hp.tile([128, CT, N], BF16, tag="h", name=f"h{s}")
                for ct in range(CT):
                    st = statp.tile([128, 2, 6], F32, tag="bnst")
                    for i in range(2):
                        nc.vector.bn_stats(st[:, i, :], x_sb[s][:, ct, i * 512:(i + 1) * 512])
                    mv = statp.tile([128, 2], F32, tag="mv")
                    nc.vector.bn_aggr(mv, st)
                    # columns (mean, E[x^2]) per partition for group aggregation
                    msq = statp.tile([128, 2], F32, tag="msq")
                    nc.vector.tensor_copy(msq[:, 0:1], mv[:, 0:1])
                    nc.vector.tensor_tensor(msq[:, 1:2], mv[:, 0:1], mv[:, 0:1], OP.mult)
                    nc.vector.tensor_tensor(msq[:, 1:2], msq[:, 1:2], mv[:, 1:2], OP.add)
                    # group-average and broadcast back to all 128 partitions (1 matmul)
                    gps = ps_sm.tile([128, 8], F32, tag="gnagg")
                    nc.tensor.matmul(gps[:, 0:2], lhsT=gmat, rhs=msq, start=True, stop=True)
                    gst = statp.tile([128, 2], F32, tag="gst")
                    nc.vector.tensor_copy(gst, gps[:, 0:2])
                    # scale = rstd * w ; shift = b - mean * scale
                    sc = statp.tile([128, 2], F32, tag="sc")
                    tmp = statp.tile([128, 1], F32, tag="tmp")
                    nc.vector.tensor_tensor(tmp, gst[:, 0:1], gst[:, 0:1], OP.mult)
                    nc.vector.tensor_tensor(tmp, gst[:, 1:2], tmp, OP.subtract)  # var
                    # rstd = exp(-0.5*ln(var+eps)); Ln+Exp live in one ACT table set
                    nc.scalar.activation(tmp, tmp, AF.Ln, bias=epsb, scale=1.0)
                    nc.scalar.activation(tmp, tmp, AF.Exp, bias=0.0, scale=-0.5)
                    nc.vector.tensor_tensor(sc[:, 0:1], tmp, nw[:, ct:ct + 1], OP.mult)
                    nc.vector.tensor_tensor(tmp, gst[:, 0:1], sc[:, 0:1], OP.mult)
                    nc.vector.tensor_tensor(sc[:, 1:2], nb[:, ct:ct + 1], tmp, OP.subtract)
                    nc.scalar.activation(h_sb[s][:, ct, :], x_sb[s][:, ct, :], AF.Identity,
                                         bias=sc[:, 1:2], scale=sc[:, 0:1])
                    # x is no longer needed raw; pre-add proj bias for the residual
                    nc.vector.tensor_scalar(x_sb[s][:, ct, :], x_sb[s][:, ct, :],
                                            pb[:, ct:ct + 1], None, OP.add)

            # ---------------- QKV ----------------
            for s in range(S):
                q_sb[s] = qp.tile([128, CT, N], BF16, tag="q", name=f"q{s}")
                k_sb[s] = kp.tile([128, CT, N], BF16, tag="k", name=f"k{s}")
                vT_sb[s] = vp.tile([128, NT, C], BF16, tag="vT", name=f"vT{s}")
                for mo in range(CT):
                    for nch in range(2):
                        ps = ps_mid.tile([128, 512], F32, tag="mid")
                        for kc in range(CT):
                            nc.tensor.matmul(ps, lhsT=wT[:, kc, mo * 128:(mo + 1) * 128],
                                             rhs=h_sb[s][:, kc, nch * 512:(nch + 1) * 512],
                                             start=(kc == 0), stop=(kc == CT - 1))
                        nc.scalar.activation(q_sb[s][:, mo, nch * 512:(nch + 1) * 512], ps,
                                             AF.Identity, bias=qb[:, mo:mo + 1], scale=1.0)
                for mo in range(CT):
                    for nch in range(2):
                        ps = ps_mid.tile([128, 512], F32, tag="mid")
                        for kc in range(CT):
                            nc.tensor.matmul(ps, lhsT=wT[:, kc, C + mo * 128:C + (mo + 1) * 128],
                                             rhs=h_sb[s][:, kc, nch * 512:(nch + 1) * 512],
                                             start=(kc == 0), stop=(kc == CT - 1))
                        nc.scalar.activation(k_sb[s][:, mo, nch * 512:(nch + 1) * 512], ps,
                                             AF.Identity, bias=kb[:, mo:mo + 1], scale=1.0)
                for it in range(NT):
                    ps = ps_mid.tile([128, 512], F32, tag="mid")
                    for kc in range(CT):
                        nc.tensor.matmul(ps, lhsT=h_sb[s][:, kc, it * 128:(it + 1) * 128],
                                         rhs=wT[:, kc, 2 * C:3 * C],
                                         start=(kc == 0), stop=(kc == CT - 1))
                    nc.vector.tensor_copy(vT_sb[s][:, it, :], ps)

            # ---------------- S^T, exp, den ----------------
            for s in range(S):
                es_sb[s] = esp.tile([128, NT, N], BF16, tag="es", name=f"es{s}")
                den_ps = ps_sm.tile([128, 8], F32, tag="den")
                nc.vector.memset(den_ps, 0.0)
                for jt in range(NT):
                    ps = ps_big.tile([128, N], F32, tag="big")
                    for nch in range(2):
                        for kc in range(CT):
                            nc.tensor.matmul(ps[:, nch * 512:(nch + 1) * 512],
                                             lhsT=k_sb[s][:, kc, jt * 128:(jt + 1) * 128],
                                             rhs=q_sb[s][:, kc, nch * 512:(nch + 1) * 512],
                                             start=(kc == 0), stop=(kc == CT - 1))
                    nc.scalar.activation(es_sb[s][:, jt, :], ps, AF.Exp, bias=0.0, scale=SCALE)
                    # den[i] += sum_j(this tile): accumulate into a memset psum bank
                    # (start=False everywhere; first write per element overwrites or
                    # adds to the zeroed data -- correct either way)
                    for ic in range(NT):
                        nc.tensor.matmul(den_ps[:, ic:ic + 1],
                                         lhsT=es_sb[s][:, jt, ic * 128:(ic + 1) * 128],
                                         rhs=ones_bf,
                                         start=False, stop=False, skip_group_check=True)
                recip[s] = statp.tile([128, 8], F32, tag="recip", name=f"recip{s}")
                nc.vector.reciprocal(recip[s], den_ps)

            # ---------------- AV (-> oT[i, c]) ----------------
            for s in range(S):
                oT_sb[s] = otp.tile([128, NT, C], F32, tag="oT", name=f"oT{s}")
                for it in range(NT):
                    ps = ps_mid.tile([128, 512], F32, tag="mid")
                    for jt in range(NT):
                        nc.tensor.matmul(ps, lhsT=es_sb[s][:, jt, it * 128:(it + 1) * 128],
                                         rhs=vT_sb[s][:, jt, :],
                                         start=(jt == 0), stop=(jt == NT - 1))
                    nc.vector.tensor_scalar(oT_sb[s][:, it, :], ps, recip[s][:, it:it + 1],
                                            None, OP.mult)

            # ---------------- transpose oT -> out[c, n] (+bv) ----------------
            for s in range(S):
                ao_sb[s] = aop.tile([128, CT, N], BF16, tag="ao", name=f"ao{s}")
                for ct in range(CT):
                    ps = ps_big.tile([128, N], F32, tag="big")
                    for it in range(NT):
                        nc.tensor.transpose(ps[:, it * 128:(it + 1) * 128],
                                            oT_sb[s][:, it, ct * 128:(ct + 1) * 128], ident)
                    nc.scalar.activation(ao_sb[s][:, ct, :], ps, AF.Identity,
                                         bias=vb[:, ct:ct + 1], scale=1.0)

            # ---------------- proj + residual ----------------
            for s in range(S):
                fin_sb[s] = finp.tile([128, CT, N], F32, tag="fin", name=f"fin{s}")
                for mo in range(CT):
                    for nch in range(2):
                        ps = ps_mid.tile([128, 512], F32, tag="mid")
                        for kc in range(CT):
                            nc.tensor.matmul(ps, lhsT=pwT[:, kc, mo * 128:(mo + 1) * 128],
                                             rhs=ao_sb[s][:, kc, nch * 512:(nch + 1) * 512],
                                             start=(kc == 0), stop=(kc == CT - 1))
                        nc.vector.tensor_tensor(fin_sb[s][:, mo, nch * 512:(nch + 1) * 512],
                                                ps, x_sb[s][:, mo, nch * 512:(nch + 1) * 512],
                                                OP.add)
                for ct in range(CT):
                    nc.sync.dma_start(out_d[s, ct * 128:(ct + 1) * 128, :], fin_sb[s][:, ct, :])

    nc.finalize()
    return nc


_NC_CACHE = None
LAST_EXEC_NS = None
LAST_RESULTS = None


def _get_nc():
    global _NC_CACHE
    if _NC_CACHE is None:
        _NC_CACHE = build_nc()
    return _NC_CACHE


def make_gmat():
    g = np.zeros((128, 128), np.float32)
    g[:64, :64] = 1.0 / 64
    g[64:, 64:] = 1.0 / 64
    return g


def make_in_maps(x, norm_w, norm_b, qkv_w, qkv_b, proj_w, proj_b):
    bf = ml_dtypes.bfloat16
    x = np.asarray(x, np.float32)
    B = x.shape[0]
    x_r = np.ascontiguousarray(x.reshape(B, C, N))
    qkv_wT = np.ascontiguousarray(np.asarray(qkv_w, np.float32).T).astype(bf)
    proj_wT = np.ascontiguousarray(np.asarray(proj_w, np.float32).T).astype(bf)
    common = {
        "qkv_wT": qkv_wT,
        "proj_wT": proj_wT,
        "norm_w": np.ascontiguousarray(np.asarray(norm_w, np.float32)),
        "norm_b": np.ascontiguousarray(np.asarray(norm_b, np.float32)),
        "qkv_b": np.ascontiguousarray(np.asarray(qkv_b, np.float32)),
        "proj_b": np.ascontiguousarray(np.asarray(proj_b, np.float32)),
        "gmat": make_gmat(),
    }
    per = B // NCORES
    return [dict(common, x=np.ascontiguousarray(x_r[c * per:(c + 1) * per]))
            for c in range(NCORES)]


def kernel(x, norm_w, norm_b, qkv_w, qkv_b, proj_w, proj_b, _trace=False):
    global LAST_EXEC_NS, LAST_RESULTS
    x = np.asarray(x)
    B, C_, H, W = x.shape
    in_maps = make_in_maps(x, norm_w, norm_b, qkv_w, qkv_b, proj_w, proj_b)
    res = run_bass_kernel_spmd(_get_nc(), in_maps, core_ids=list(range(NCORES)),
                               trace=_trace)
    LAST_EXEC_NS = res.exec_time_ns
    LAST_RESULTS = res
    out = np.concatenate([res.results[c]["out"] for c in range(NCORES)], axis=0)
    return out.reshape(B, C_, H, W).astype(np.float32)
